# revision 17
# baseline (speedup 1.0000x reference)
"""Trainium2 Bass kernel for nn_CausalAttention (B=4, T=2048, d_model=1024, d_ff=2048).

Sharding: 8 cores = 4 batches x 2 pair-halves. Each core owns 8 query blocks
of 128 rows (OWN_H), paired so causal work is balanced and the per-core
program is IDENTICAL (SPMD): the k-th owned block always computes E[k] key
chunks; exact causal masking arrives as per-core input data.

Math per core (fp32r = 4-byte fp32 storage, ~tf32 matmul precision, 1 PE
cycle/row at >=256-wide moving vs fp32's 4 - measured max|err| 0.028 on
score-scale tiles vs bf16's 0.53; bf16 is 0.53 and corrupts rows):
  M  = Wq @ Wk.T        (fp32r; c1-row slice per core, AllGather -> full M)
  N2 = Wv @ Wf          (bf16; g-slice per core, half-AllGather by g-half)
  vf = x @ N2[:, my g-half]   (bf16, FULL T; stays entirely in SBUF)
  uT = (xq M).T         (fp32r; owned query rows only)
  S  = uT.T @ x.T == q @ k.T  (fp32r scores; contracts d_model=1024 not d_ff)
  P' = softmax(S + mask) / rowsum  (exp on ScalarE, rescaled bf16 probs)
  out[:, my g-half] = P'm @ vf + bf[g-half]   for BOTH pair members m

Key structure: instead of AllGathering vf rows (32 MB of DMA), the pair
exchanges transposed prob tiles P' (2.4 MB): each core keeps its g-half of
vf in SBUF and computes the output column-half for both members' queries.
The P' AllGather is split in two (big-E blocks first) so the out phase
starts as soon as the first half lands. Output is written bf16 as
[member, k, 128, g-half]; the host reassembles rows x column-halves.

DMA FIFO discipline (two HWDGE rings): nc.sync carries input loads and
collective-gated reads in strictly increasing gate order (n2h -> mall ->
pall); nc.scalar carries the small collective-input writes (n2s, msl, pts)
and output writes, so AllGather triggers are never head-of-line blocked
behind megabyte loads. vf has no DRAM traffic at all.
"""

import sys
from contextlib import ExitStack

for _p in ("/opt/trn_rl_repo", "/root/.axon_site/_ro/trn_rl_repo"):
    if _p not in sys.path:
        sys.path.append(_p)

import ml_dtypes
import numpy as np

import concourse.bass as bass
import concourse.mybir as mybir
import concourse.tile as tile
from concourse import bacc
from concourse.bass_utils import run_bass_kernel_spmd
from concourse.masks import make_identity

F32 = mybir.dt.float32
F32R = mybir.dt.float32r
BF16 = mybir.dt.bfloat16

B, T, C, F = 4, 2048, 1024, 2048
NB = T // 128  # 16 query/key blocks per batch
CC = C // 128  # 8 chunks of d_model
FC = F // 128  # 16 chunks of d_ff
NCORES = 8

# k-th owned block of each half; chosen so L(OWN_H[h][k]) <= E[k] for both h
# and sum(E)=72 (ideal causal: 68). E[k] = key chunks computed for block k.
OWN_H = {
    0: [15, 12, 11, 8, 7, 4, 3, 0],
    1: [14, 13, 10, 9, 6, 5, 2, 1],
}
E = [16, 14, 12, 10, 8, 6, 4, 2]
EOFF = [0, 16, 30, 42, 52, 60, 66, 70]  # prefix sums of E (ptsb tile offsets)
NP1 = EOFF[4]  # 52 tiles in the first P AllGather (k=0..3)
NP2 = EOFF[7] + E[7] - NP1  # 20 tiles in the second (k=4..7)
NEG = -1.0e30

ALL8 = [list(range(8))]
PAIRS = [[0, 1], [2, 3], [4, 5], [6, 7]]
HALVES = [[0, 2, 4, 6], [1, 3, 5, 7]]  # g-half gather groups (h = core % 2)
WAVES = [(0, 1), (2, 3), (4, 5), (6, 7)]  # big E first -> small tail

_CACHE = {}


def _build_program():
    """Trace + finalize the (single, SPMD) Bass program."""
    nc = bacc.Bacc(None)

    # all operands arrive pre-transposed / pre-cast / pre-sliced from the host
    xT_ext = nc.declare_dram_parameter("xTin", [C, T], F32R, isOutput=False)
    xqT_ext = nc.declare_dram_parameter("xqTin", [C, 1024], F32R, isOutput=False)
    xvT_ext = nc.declare_dram_parameter("xvTb", [C, T], BF16, isOutput=False)
    m2_ext = nc.declare_dram_parameter("mask2", [8, 128, 256], F32, isOutput=False)
    wkT_ext = nc.declare_dram_parameter("WkT", [F, C], F32R, isOutput=False)
    wqs_ext = nc.declare_dram_parameter("WqTs", [F, 128], F32R, isOutput=False)
    wvT_ext = nc.declare_dram_parameter("WvTb", [F, C], BF16, isOutput=False)
    wfs_ext = nc.declare_dram_parameter("Wfs", [F, 256], BF16, isOutput=False)
    bfh_ext = nc.declare_dram_parameter("bfh", [1024], F32, isOutput=False)
    out_ext = nc.declare_dram_parameter("out", [2, 8, 128, 1024], BF16, isOutput=True)

    with tile.TileContext(nc) as tc, ExitStack() as root:
        persist = root.enter_context(tc.tile_pool(name="persist", bufs=1))
        dram = root.enter_context(tc.tile_pool(name="dram", bufs=1, space="DRAM"))

        identbf = persist.tile([128, 128], BF16, tag="identbf")
        make_identity(nc, identbf[:, :])
        # long-lived operands (loads emitted late, where first needed)
        xT = persist.tile([128, CC, T], F32R, tag="xT")  # 64KB/part
        uT = persist.tile([128, CC, 1024], F32R, tag="uT")  # 32KB/part

        # collective buffers (DRAM)
        msl_d = dram.tile([128, C], F32R, tag="msl_d")
        mall_d = dram.tile([NCORES * 128, C], F32R, tag="mall_d", addr_space="Shared")
        n2s_d = dram.tile([C, 256], BF16, tag="n2s_d")
        n2h_d = dram.tile([4, CC, 128, 256], BF16, tag="n2h_d")
        pts_d = dram.tile([EOFF[7] + E[7], 128, 128], BF16, tag="pts_d")
        pall1_d = dram.tile([2, NP1, 128, 128], BF16, tag="pall1_d")
        pall2_d = dram.tile([2, NP2, 128, 128], BF16, tag="pall2_d")

        # ======== phase 1: N2-slice = Wv @ Wf[:, my g 256 cols], half-AG =====
        with ExitStack() as ph1:
            wvp = ph1.enter_context(tc.tile_pool(name="wvp", bufs=1))
            ps1 = ph1.enter_context(tc.tile_pool(name="ps1", bufs=1, space="PSUM"))
            wvT = wvp.tile([128, FC, C], BF16, tag="wvT")  # 32KB/part
            wfs = wvp.tile([128, FC, 256], BF16, tag="wfs")  # 8KB/part
            for f in range(FC):  # interleaved so f=0 operands arrive first
                nc.sync.dma_start(
                    out=wfs[:, f, :], in_=wfs_ext[f * 128 : (f + 1) * 128, :]
                )
                nc.sync.dma_start(
                    out=wvT[:, f, :], in_=wvT_ext[f * 128 : (f + 1) * 128, :]
                )
            n2s_sb = wvp.tile([128, CC, 256], BF16, tag="n2s_sb")  # 4KB/part
            for ah in range(2):
                nps = ps1.tile([128, 4, 512], F32, tag="nps", name=f"nps{ah}")
                for f in range(FC):
                    for a4 in range(4):
                        nc.tensor.matmul(
                            nps[:, a4, :256],
                            wvT[:, f, (ah * 4 + a4) * 128 : (ah * 4 + a4 + 1) * 128],
                            wfs[:, f, :],
                            start=(f == 0),
                            stop=(f == FC - 1),
                        )
                for a4 in range(4):
                    nc.vector.tensor_copy(
                        out=n2s_sb[:, ah * 4 + a4, :], in_=nps[:, a4, :256]
                    )
            for a in range(CC):  # scalar FIFO: not blocked behind wkT load
                nc.scalar.dma_start(
                    out=n2s_d[a * 128 : (a + 1) * 128, :], in_=n2s_sb[:, a, :]
                )
            # gather my g-half: 4 slices of 256 across my half-group
            nc.gpsimd.collective_compute(
                "AllGather",
                mybir.AluOpType.bypass,
                replica_groups=HALVES,
                ins=[n2s_d[:, :]],
                outs=[n2h_d[:, :, :, :]],
            )

        # ======== phase 2: M-slice = Wq.T[my c1 128 rows] @ WkT, AllGather ===
        with ExitStack() as ph2:
            wqp = ph2.enter_context(tc.tile_pool(name="wqp", bufs=1))
            ps2 = ph2.enter_context(tc.tile_pool(name="ps2", bufs=1, space="PSUM"))
            wkT = wqp.tile([128, FC, C], F32R, tag="wkT")  # 64KB/part
            wqs = wqp.tile([128, FC, 128], F32R, tag="wqs")  # 8KB/part
            for f in range(FC):
                nc.sync.dma_start(
                    out=wqs[:, f, :], in_=wqs_ext[f * 128 : (f + 1) * 128, :]
                )
                nc.sync.dma_start(
                    out=wkT[:, f, :], in_=wkT_ext[f * 128 : (f + 1) * 128, :]
                )
            msl_sb = wqp.tile([128, C], F32R, tag="msl_sb")  # 4KB/part
            mps = ps2.tile([128, 2, 512], F32, tag="mps")  # 2 banks
            for f in range(FC):
                for ch in range(2):
                    nc.tensor.matmul(
                        mps[:, ch, :],
                        wqs[:, f, :],
                        wkT[:, f, ch * 512 : (ch + 1) * 512],
                        start=(f == 0),
                        stop=(f == FC - 1),
                    )
            nc.vector.tensor_copy(out=msl_sb[:, :], in_=mps[:, :, :])
            nc.scalar.dma_start(out=msl_d[:, :], in_=msl_sb[:, :])
            nc.gpsimd.collective_compute(
                "AllGather",
                mybir.AluOpType.bypass,
                replica_groups=ALL8,
                ins=[msl_d[:, :]],
                outs=[mall_d[:, :]],
            )

        # pool spanning vf .. out (vf half + transposed probs + bias + masks)
        ph36 = root.enter_context(ExitStack())
        atp = ph36.enter_context(tc.tile_pool(name="atp", bufs=1))
        vfo = atp.tile([128, NB, 1024], BF16, tag="vfo")  # 32KB/part
        ptsb = atp.tile([128, EOFF[7] + E[7], 128], BF16, tag="ptsb")  # 18KB
        m2 = atp.tile([128, 8, 256], F32, tag="m2")  # 8KB/part
        bfh = atp.tile([128, 1024], F32, tag="bfh")  # 4KB/part

        # ======== phase 3: vf-half = x[all T] @ N2[:, my g-half], in SBUF ====
        with ExitStack() as ph3:
            n2p = ph3.enter_context(tc.tile_pool(name="n2p", bufs=1))
            ps3 = ph3.enter_context(tc.tile_pool(name="ps3", bufs=2, space="PSUM"))
            xvb = n2p.tile([128, CC, T], BF16, tag="xvb")  # 32KB/part
            for cc in range(CC):
                nc.sync.dma_start(
                    out=xvb[:, cc, :], in_=xvT_ext[cc * 128 : (cc + 1) * 128, :]
                )
            bfh_ap = bfh_ext[:]
            nc.sync.dma_start(
                out=bfh,
                in_=bass.AP(
                    tensor=bfh_ap.tensor,
                    offset=bfh_ap.offset,
                    ap=[[0, 128]] + list(bfh_ap.ap),
                ),
            )
            for k in range(8):
                nc.sync.dma_start(out=m2[:, k, :], in_=m2_ext[k])
            n2h = n2p.tile([128, CC, 1024], BF16, tag="n2h")  # 16KB/part
            # n2h_d rows [pos 4][cc][p][256] -> n2h[p, cc, pos*256:...]
            for gi in range(4):
                nc.sync.dma_start(
                    out=n2h[:, :, gi * 256 : (gi + 1) * 256],
                    in_=n2h_d[gi].transpose([1, 0, 2]),
                )
            for tb in range(NB):
                vps = ps3.tile([128, 1024], F32, tag="vps", name=f"vps{tb}")
                for cc in range(CC):  # stationary reused across both movings
                    for g2 in range(2):
                        nc.tensor.matmul(
                            vps[:, g2 * 512 : (g2 + 1) * 512],
                            xvb[:, cc, tb * 128 : (tb + 1) * 128],
                            n2h[:, cc, g2 * 512 : (g2 + 1) * 512],
                            start=(cc == 0),
                            stop=(cc == CC - 1),
                        )
                nc.vector.tensor_copy(out=vfo[:, tb, :], in_=vps[:, :])

        # ======== phase 4: uT = (xq M).T  [c2-chunk, owned-t] fp32r ==========
        # mM is streamed per c1-chunk (2-deep) to fit SBUF; two passes over
        # mall (tt halves) with all 8 c2 accumulators resident (8 PSUM banks)
        with ExitStack() as ph4:
            mxp = ph4.enter_context(tc.tile_pool(name="mxp", bufs=1))
            mmp = ph4.enter_context(tc.tile_pool(name="mmp", bufs=2))
            ps4 = ph4.enter_context(tc.tile_pool(name="ps4", bufs=1, space="PSUM"))
            xqT = mxp.tile([128, CC, 1024], F32R, tag="xqT")  # 32KB/part
            for cc in range(CC):
                nc.sync.dma_start(
                    out=xqT[:, cc, :], in_=xqT_ext[cc * 128 : (cc + 1) * 128, :]
                )
            for tt in range(2):
                ups8 = ps4.tile([128, CC, 512], F32, tag="ups8", name=f"ups8_{tt}")
                for c1 in range(CC):
                    mM = mmp.tile([128, C], F32R, tag="mM", name=f"mM{tt}_{c1}")
                    nc.sync.dma_start(
                        out=mM[:, :], in_=mall_d[c1 * 128 : (c1 + 1) * 128, :]
                    )
                    for c2 in range(CC):
                        nc.tensor.matmul(
                            ups8[:, c2, :],
                            mM[:, c2 * 128 : (c2 + 1) * 128],
                            xqT[:, c1, tt * 512 : (tt + 1) * 512],
                            start=(c1 == 0),
                            stop=(c1 == CC - 1),
                        )
                for c2 in range(CC):
                    nc.vector.tensor_copy(
                        out=uT[:, c2, tt * 512 : (tt + 1) * 512], in_=ups8[:, c2, :]
                    )
            # xT loads: sync FIFO right after uT operands; needed by phase 5a
            for cc in range(CC):
                nc.sync.dma_start(
                    out=xT[:, cc, :], in_=xT_ext[cc * 128 : (cc + 1) * 128, :]
                )

        # ======== phase 5a: scores + softmax + rescaled transposed probs =====
        with ExitStack() as ph5:
            st5 = ph5.enter_context(tc.tile_pool(name="st5", bufs=2))
            small = ph5.enter_context(tc.tile_pool(name="small", bufs=4))
            ps5 = ph5.enter_context(tc.tile_pool(name="ps5", bufs=2, space="PSUM"))
            ps_t = ph5.enter_context(tc.tile_pool(name="ps_t", bufs=2, space="PSUM"))

            def softmax_stage(k):
                """scores -> masked SBUF copy -> exp -> P/rowsum bf16"""
                ek = E[k]
                scols = ek * 128
                s_sb = st5.tile([128, T], F32, tag="s_sb", name=f"s_sb{k}")
                for h0 in range(0, scols, 1024):
                    hw = min(1024, scols - h0)
                    sps = ps5.tile([128, 1024], F32, tag="sps", name=f"sps{k}_{h0}")
                    for st in range(0, hw, 512):
                        w = min(512, hw - st)
                        for c2 in range(CC):
                            nc.tensor.matmul(
                                sps[:, st : st + w],
                                uT[:, c2, k * 128 : (k + 1) * 128],
                                xT[:, c2, h0 + st : h0 + st + w],
                                start=(c2 == 0),
                                stop=(c2 == CC - 1),
                            )
                    m0 = scols - 256  # mask window start
                    plain = min(hw, max(0, m0 - h0))
                    if plain > 0:
                        nc.vector.tensor_copy(
                            out=s_sb[:, h0 : h0 + plain], in_=sps[:, :plain]
                        )
                    if plain < hw:
                        nc.vector.tensor_add(
                            s_sb[:, h0 + plain : h0 + hw],
                            sps[:, plain:hw],
                            m2[:, k, h0 + plain - m0 : h0 + hw - m0],
                        )
                negmax = small.tile(
                    [128, 1], F32, tag="negmax", name=f"negmax{k}", bufs=6
                )
                nc.vector.tensor_reduce(
                    out=negmax,
                    in_=s_sb[:, :scols],
                    axis=mybir.AxisListType.X,
                    op=mybir.AluOpType.max,
                    negate=True,
                )
                psb = st5.tile([128, T], BF16, tag="psb", name=f"psb{k}", bufs=3)
                rsum = small.tile([128, 1], F32, tag="rsum", name=f"rsum{k}", bufs=6)
                nc.scalar.activation(
                    out=psb[:, :scols],
                    in_=s_sb[:, :scols],
                    func=mybir.ActivationFunctionType.Exp,
                    bias=negmax,
                    scale=1.0,
                    accum_out=rsum,
                )
                rinv = small.tile([128, 1], F32, tag="rinv", name=f"rinv{k}", bufs=6)
                nc.vector.reciprocal(out=rinv, in_=rsum)
                psbS = st5.tile([128, T], BF16, tag="psbS", name=f"psbS{k}", bufs=3)
                nc.vector.tensor_scalar_mul(psbS[:, :scols], psb[:, :scols], rinv)
                return psbS

            def transpose_stage(k, psbS):
                for sc in range(E[k]):
                    pt = ps_t.tile([128, 128], BF16, tag="pt", name=f"pt{k}_{sc}")
                    nc.tensor.transpose(
                        pt[:, :], psbS[:, sc * 128 : (sc + 1) * 128], identbf[:, :]
                    )
                    nc.vector.tensor_copy(out=ptsb[:, EOFF[k] + sc, :], in_=pt[:, :])
                # stream P' tiles to DRAM for the pair exchange (scalar FIFO)
                o = EOFF[k]
                nc.scalar.dma_start(
                    out=pts_d[o : o + E[k]].transpose([1, 0, 2]),
                    in_=ptsb[:, o : o + E[k], :],
                )

            staged = {0: softmax_stage(0)}
            for k in range(8):
                if k + 1 < 8:
                    staged[k + 1] = softmax_stage(k + 1)
                transpose_stage(k, staged.pop(k))
                if k == 3:  # first 52 tiles ready: ship the big-E half early
                    nc.gpsimd.collective_compute(
                        "AllGather",
                        mybir.AluOpType.bypass,
                        replica_groups=PAIRS,
                        ins=[pts_d[:NP1]],
                        outs=[pall1_d[:, :, :, :]],
                    )
            nc.gpsimd.collective_compute(
                "AllGather",
                mybir.AluOpType.bypass,
                replica_groups=PAIRS,
                ins=[pts_d[NP1:]],
                outs=[pall2_d[:, :, :, :]],
            )

        # ======== phase 5b: out[:, my g-half] = P'm @ vf for both members ====
        with ExitStack() as ph6:
            prd = ph6.enter_context(tc.tile_pool(name="prd", bufs=2))
            orp = ph6.enter_context(tc.tile_pool(name="orp", bufs=2))
            ps6 = ph6.enter_context(tc.tile_pool(name="ps6", bufs=2, space="PSUM"))

            def pread(m, k, name):
                e = E[k]
                t = prd.tile([128, 16, 128], BF16, tag="pk", name=name, bufs=4)
                if k < 4:
                    src = pall1_d[m, EOFF[k] : EOFF[k] + e]
                else:
                    src = pall2_d[m, EOFF[k] - NP1 : EOFF[k] - NP1 + e]
                nc.sync.dma_start(out=t[:, :e, :], in_=src.transpose([1, 0, 2]))
                return t

            def epilogue(m, k, ops):
                orow = orp.tile([128, 1024], BF16, tag="orow", name=f"or{m}_{k}")
                nc.vector.tensor_add(orow, ops, bfh)
                nc.scalar.dma_start(out=out_ext[m, k], in_=orow)

            for m in range(2):  # pair member whose queries we compute
                for ka, kb in WAVES:
                    ea, eb = E[ka], E[kb]
                    pka = pread(m, ka, f"pka{m}_{ka}")
                    pkb = pread(m, kb, f"pkb{m}_{kb}")
                    opsA = ps6.tile([128, 1024], F32, tag="opsA", name=f"oA{m}{ka}")
                    opsB = ps6.tile([128, 1024], F32, tag="opsB", name=f"oB{m}{kb}")
                    for sc in range(ea):
                        for g2 in range(2):
                            nc.tensor.matmul(
                                opsA[:, g2 * 512 : (g2 + 1) * 512],
                                pka[:, sc, :],
                                vfo[:, sc, g2 * 512 : (g2 + 1) * 512],
                                start=(sc == 0),
                                stop=(sc == ea - 1),
                            )
                        if sc < eb:
                            for g2 in range(2):
                                nc.tensor.matmul(
                                    opsB[:, g2 * 512 : (g2 + 1) * 512],
                                    pkb[:, sc, :],
                                    vfo[:, sc, g2 * 512 : (g2 + 1) * 512],
                                    start=(sc == 0),
                                    stop=(sc == eb - 1),
                                )
                        if sc == eb:
                            epilogue(m, kb, opsB)  # B stopped; drain while A runs
                    epilogue(m, ka, opsA)

    nc.finalize()
    return nc


def _get_program():
    if "nc" not in _CACHE:
        _CACHE["nc"] = _build_program()
    return _CACHE["nc"]


def _make_in_maps(x, Wq, Wk, Wv, Wf, bf):
    x = np.ascontiguousarray(x, dtype=np.float32)
    WqT = np.ascontiguousarray(np.asarray(Wq, dtype=np.float32).T)
    WkT = np.ascontiguousarray(np.asarray(Wk, dtype=np.float32).T)
    WvTb = np.ascontiguousarray(np.asarray(Wv, dtype=np.float32).T).astype(
        ml_dtypes.bfloat16
    )
    Wfb = np.asarray(Wf, dtype=np.float32).astype(ml_dtypes.bfloat16)
    bf = np.ascontiguousarray(bf, dtype=np.float32)
    in_maps = []
    for core in range(NCORES):
        b, h = core // 2, core % 2
        own = OWN_H[h]
        xb = x[b]
        xbT = np.ascontiguousarray(xb.T)
        xq = np.concatenate([xb[blk * 128 : (blk + 1) * 128] for blk in own], axis=0)
        mask2 = np.zeros((8, 128, 256), dtype=np.float32)
        for k, blk in enumerate(own):
            s0 = (E[k] - 2) * 128  # global key index of mask window start
            s = s0 + np.arange(256)[None, :]
            t = blk * 128 + np.arange(128)[:, None]
            mask2[k] = np.where(s <= t, 0.0, NEG).astype(np.float32)
        gidx = h * 4 + b  # my Wf g-slice; group [h::2] holds g-half h in order
        in_maps.append(
            {
                "xTin": xbT,
                "xqTin": np.ascontiguousarray(xq.T),
                "xvTb": xbT.astype(ml_dtypes.bfloat16),
                "mask2": mask2,
                "WkT": WkT,
                "WqTs": np.ascontiguousarray(WqT[:, core * 128 : (core + 1) * 128]),
                "WvTb": WvTb,
                "Wfs": np.ascontiguousarray(Wfb[:, gidx * 256 : (gidx + 1) * 256]),
                "bfh": np.ascontiguousarray(bf[h * 1024 : (h + 1) * 1024]),
            }
        )
    return in_maps


def run_on_hw(inputs, trace=False, trace_cores=None):
    nc = _get_program()
    in_maps = _make_in_maps(**inputs)
    res = run_bass_kernel_spmd(
        nc, in_maps, list(range(NCORES)), trace=trace, trace_cores=trace_cores
    )
    out = np.empty((B, T, F), dtype=np.float32)
    for core in range(NCORES):
        b, h = core // 2, core % 2
        o = res.results[core]["out"]  # [2, 8, 128, 1024] bf16
        for m in range(2):
            for k, blk in enumerate(OWN_H[m]):
                out[b, blk * 128 : (blk + 1) * 128, h * 1024 : (h + 1) * 1024] = o[
                    m, k
                ].astype(np.float32)
    return out, res


def kernel(x, Wq, Wk, Wv, Wf, bf):
    out, _ = run_on_hw(dict(x=x, Wq=Wq, Wk=Wk, Wv=Wv, Wf=Wf, bf=bf))
    return out


# revision 20
# speedup vs baseline: 1.1031x; 1.1031x over previous
"""Trainium2 Bass kernel for nn_CausalAttention (B=4, T=2048, d_model=1024, d_ff=2048).

Sharding: 8 cores = 4 batches x 2 pair-halves. Each core owns 8 query blocks
of 128 rows (OWN_H), paired so causal work is balanced and the per-core
program is IDENTICAL (SPMD): the k-th owned block always computes E[k] key
chunks; exact causal masking arrives as per-core input data.

Math per core (fp32r = 4-byte fp32 storage, ~tf32 matmul precision, 1 PE
cycle/row at >=256-wide moving vs fp32's 4 - measured max|err| 0.028 on
score-scale tiles; bf16 is 0.53 and corrupts rows):
  M  = Wq @ Wk.T        (fp32r; c1-row slice per core, AllGather -> full M)
  N2 = Wv @ Wf          (bf16; g-slice per core, half-AllGather by g-half)
  uT = (xq M).T         (fp32r; owned query rows only)
  S  = uT.T @ x.T == q @ k.T  (fp32r scores; contracts d_model=1024 not d_ff)
  P' = softmax(S + mask) / rowsum  (exp on ScalarE, rescaled bf16 probs)
  vf = x @ N2[:, my g-half]   (bf16, FULL T; stays entirely in SBUF)
  out[:, my g-half] = P'm @ vf + bf[g-half]   for BOTH pair members m

Key structures:
- Instead of AllGathering vf rows (32 MB of DMA), the pair exchanges
  transposed prob tiles P' (2.4 MB): each core keeps its g-half of vf in
  SBUF and computes the output column-half for both members' queries.
  Output is written bf16 as [member, k, 128, g-half]; host reassembles.
- fp32r stationary loads cannot overlap the running matmul (the 4-byte
  weight load is folded into the MATMUL), so fp32r loops are ordered to
  reuse each stationary across as many moving matmuls as possible
  (uT: 2x, S: up to 4x).
- Collectives serialize globally (next mesh begins only after the previous
  one COMPLETES, ~4-18us after mesh end), so their order mirrors need
  order and AG-M is split in c2-halves to release uT earlier; the PE order
  uT -> scores -> vf -> out lets the vf matmuls cover the P exchange.

DMA FIFO discipline (two HWDGE rings, FIFO per ring, order = emission
order): nc.sync carries input loads and collective-gated reads with
monotone gate times (mall-h1 -> mall-h2 -> n2h -> pall); nc.scalar carries
the small collective-input writes (msl, n2s, pts) and output writes, so
AllGather triggers are never head-of-line blocked behind megabyte loads.
"""

import sys
from contextlib import ExitStack

for _p in ("/opt/trn_rl_repo", "/root/.axon_site/_ro/trn_rl_repo"):
    if _p not in sys.path:
        sys.path.append(_p)

import ml_dtypes
import numpy as np

import concourse.bass as bass
import concourse.mybir as mybir
import concourse.tile as tile
from concourse import bacc
from concourse.bass_utils import run_bass_kernel_spmd
from concourse.masks import make_identity

F32 = mybir.dt.float32
F32R = mybir.dt.float32r
BF16 = mybir.dt.bfloat16

B, T, C, F = 4, 2048, 1024, 2048
NB = T // 128  # 16 query/key blocks per batch
CC = C // 128  # 8 chunks of d_model
FC = F // 128  # 16 chunks of d_ff
NCORES = 8

# k-th owned block of each half; chosen so L(OWN_H[h][k]) <= E[k] for both h
# and sum(E)=72 (ideal causal: 68). E[k] = key chunks computed for block k.
OWN_H = {
    0: [15, 12, 11, 8, 7, 4, 3, 0],
    1: [14, 13, 10, 9, 6, 5, 2, 1],
}
E = [16, 14, 12, 10, 8, 6, 4, 2]
EOFF = [0, 16, 30, 42, 52, 60, 66, 70]  # prefix sums of E (ptsb tile offsets)
NPT = EOFF[7] + E[7]  # 72 transposed prob tiles
NEG = -1.0e30

ALL8 = [list(range(8))]
PAIRS = [[0, 1], [2, 3], [4, 5], [6, 7]]
HALVES = [[0, 2, 4, 6], [1, 3, 5, 7]]  # g-half gather groups (h = core % 2)
WAVES = [(0, 1), (2, 3), (4, 5), (6, 7)]  # big E first -> small tail

_CACHE = {}


def _build_program():
    """Trace + finalize the (single, SPMD) Bass program."""
    nc = bacc.Bacc(None)

    # all operands arrive pre-transposed / pre-cast / pre-sliced from the host
    xT_ext = nc.declare_dram_parameter("xTin", [C, T], F32R, isOutput=False)
    xqT_ext = nc.declare_dram_parameter("xqTin", [C, 1024], F32R, isOutput=False)
    xvT_ext = nc.declare_dram_parameter("xvTb", [C, T], BF16, isOutput=False)
    m2_ext = nc.declare_dram_parameter("mask2", [8, 128, 256], F32, isOutput=False)
    wkT_ext = nc.declare_dram_parameter("WkT", [F, C], F32R, isOutput=False)
    wqs_ext = nc.declare_dram_parameter("WqTs", [F, 128], F32R, isOutput=False)
    wvT_ext = nc.declare_dram_parameter("WvTb", [F, C], BF16, isOutput=False)
    wfs_ext = nc.declare_dram_parameter("Wfs", [F, 256], BF16, isOutput=False)
    bfh_ext = nc.declare_dram_parameter("bfh", [1024], F32, isOutput=False)
    out_ext = nc.declare_dram_parameter("out", [2, 8, 128, 1024], BF16, isOutput=True)

    with tile.TileContext(nc) as tc, ExitStack() as root:
        persist = root.enter_context(tc.tile_pool(name="persist", bufs=1))
        dram = root.enter_context(tc.tile_pool(name="dram", bufs=1, space="DRAM"))

        identbf = persist.tile([128, 128], BF16, tag="identbf")
        make_identity(nc, identbf[:, :])
        # long-lived operands (loads emitted late, where first needed)
        xT = persist.tile([128, CC, T], F32R, tag="xT")  # 64KB/part
        uT = persist.tile([128, CC, 1024], F32R, tag="uT")  # 32KB/part

        # collective buffers (DRAM)
        mslA_d = dram.tile([128, 512], F32R, tag="mslA_d")
        mslB_d = dram.tile([128, 512], F32R, tag="mslB_d")
        mallA_d = dram.tile([CC, 128, 512], F32R, tag="mallA_d", addr_space="Shared")
        mallB_d = dram.tile([CC, 128, 512], F32R, tag="mallB_d", addr_space="Shared")
        n2s_d = dram.tile([C, 256], BF16, tag="n2s_d")
        n2h_d = dram.tile([4, CC, 128, 256], BF16, tag="n2h_d")
        pts_d = dram.tile([NPT, 128, 128], BF16, tag="pts_d")
        pall_d = dram.tile([2, NPT, 128, 128], BF16, tag="pall_d")

        # ======== phase 1: M-slice = Wq.T[my c1 128 rows] @ WkT ==============
        # out [c1-128, c2-1024] fp32r at 512-wide moving; AllGather in two
        # c2-halves so uT can start on c2 0-3 one mesh earlier.
        with ExitStack() as ph1:
            wqp = ph1.enter_context(tc.tile_pool(name="wqp", bufs=1))
            ps1 = ph1.enter_context(tc.tile_pool(name="ps1", bufs=1, space="PSUM"))
            wkT = wqp.tile([128, FC, C], F32R, tag="wkT")  # 64KB/part
            wqs = wqp.tile([128, FC, 128], F32R, tag="wqs")  # 8KB/part
            for f in range(FC):  # interleaved so f=0 operands arrive first
                nc.sync.dma_start(
                    out=wqs[:, f, :], in_=wqs_ext[f * 128 : (f + 1) * 128, :]
                )
                nc.sync.dma_start(
                    out=wkT[:, f, :], in_=wkT_ext[f * 128 : (f + 1) * 128, :]
                )
            msl_sb = wqp.tile([128, C], F32R, tag="msl_sb")  # 4KB/part
            mps = ps1.tile([128, 2, 512], F32, tag="mps")  # 2 banks
            for f in range(FC):
                for ch in range(2):
                    nc.tensor.matmul(
                        mps[:, ch, :],
                        wqs[:, f, :],
                        wkT[:, f, ch * 512 : (ch + 1) * 512],
                        start=(f == 0),
                        stop=(f == FC - 1),
                    )
            nc.vector.tensor_copy(out=msl_sb[:, :], in_=mps[:, :, :])
            nc.scalar.dma_start(out=mslA_d[:, :], in_=msl_sb[:, :512])
            nc.scalar.dma_start(out=mslB_d[:, :], in_=msl_sb[:, 512:])
            nc.gpsimd.collective_compute(
                "AllGather",
                mybir.AluOpType.bypass,
                replica_groups=ALL8,
                ins=[mslA_d[:, :]],
                outs=[mallA_d[:, :, :]],
            )
            nc.gpsimd.collective_compute(
                "AllGather",
                mybir.AluOpType.bypass,
                replica_groups=ALL8,
                ins=[mslB_d[:, :]],
                outs=[mallB_d[:, :, :]],
            )

        # ======== phase 2: N2-slice = Wv @ Wf[:, my g 256 cols], half-AG =====
        with ExitStack() as ph2:
            wvp = ph2.enter_context(tc.tile_pool(name="wvp", bufs=1))
            ps2 = ph2.enter_context(tc.tile_pool(name="ps2", bufs=1, space="PSUM"))
            wvT = wvp.tile([128, FC, C], BF16, tag="wvT")  # 32KB/part
            wfs = wvp.tile([128, FC, 256], BF16, tag="wfs")  # 8KB/part
            for f in range(FC):
                nc.sync.dma_start(
                    out=wfs[:, f, :], in_=wfs_ext[f * 128 : (f + 1) * 128, :]
                )
                nc.sync.dma_start(
                    out=wvT[:, f, :], in_=wvT_ext[f * 128 : (f + 1) * 128, :]
                )
            n2s_sb = wvp.tile([128, CC, 256], BF16, tag="n2s_sb")  # 4KB/part
            for ah in range(2):
                nps = ps2.tile([128, 4, 512], F32, tag="nps", name=f"nps{ah}")
                for f in range(FC):
                    for a4 in range(4):
                        nc.tensor.matmul(
                            nps[:, a4, :256],
                            wvT[:, f, (ah * 4 + a4) * 128 : (ah * 4 + a4 + 1) * 128],
                            wfs[:, f, :],
                            start=(f == 0),
                            stop=(f == FC - 1),
                        )
                for a4 in range(4):
                    nc.vector.tensor_copy(
                        out=n2s_sb[:, ah * 4 + a4, :], in_=nps[:, a4, :256]
                    )
            for a in range(CC):  # scalar FIFO: not blocked behind loads
                nc.scalar.dma_start(
                    out=n2s_d[a * 128 : (a + 1) * 128, :], in_=n2s_sb[:, a, :]
                )
            # gather my g-half: 4 slices of 256 across my half-group
            nc.gpsimd.collective_compute(
                "AllGather",
                mybir.AluOpType.bypass,
                replica_groups=HALVES,
                ins=[n2s_d[:, :]],
                outs=[n2h_d[:, :, :, :]],
            )

        # pool spanning 5a .. out (vf half + transposed probs)
        atp = root.enter_context(tc.tile_pool(name="atp", bufs=1))
        vfo = atp.tile([128, NB, 1024], BF16, tag="vfo")  # 32KB/part
        ptsb = atp.tile([128, NPT, 128], BF16, tag="ptsb")  # 18KB/part

        # xqT load next on the sync FIFO (ungated; needed first by uT)
        xq_p = root.enter_context(ExitStack())
        xqp = xq_p.enter_context(tc.tile_pool(name="xqp", bufs=1))
        xqT = xqp.tile([128, CC, 1024], F32R, tag="xqT")  # 32KB/part
        for cc in range(CC):
            nc.sync.dma_start(
                out=xqT[:, cc, :], in_=xqT_ext[cc * 128 : (cc + 1) * 128, :]
            )

        # ======== phase 3: uT = (xq M).T  [c2-chunk, owned-t] fp32r ==========
        # M streamed per c2-chunk; each fp32r stationary feeds both tt movings
        with ExitStack() as ph3:
            mmp = ph3.enter_context(tc.tile_pool(name="mmp", bufs=2))
            ps3 = ph3.enter_context(tc.tile_pool(name="ps3", bufs=2, space="PSUM"))
            for c2 in range(CC):
                mMc = mmp.tile([128, CC, 128], F32R, tag="mMc", name=f"mMc{c2}")
                half, lo = (mallA_d, 0) if c2 < 4 else (mallB_d, 4)
                nc.sync.dma_start(
                    out=mMc[:, :, :],
                    in_=half[:, :, (c2 - lo) * 128 : (c2 - lo + 1) * 128].transpose(
                        [1, 0, 2]
                    ),
                )
                ups2 = ps3.tile([128, 2, 512], F32, tag="ups2", name=f"ups2_{c2}")
                for c1 in range(CC):
                    for tt in range(2):
                        nc.tensor.matmul(
                            ups2[:, tt, :],
                            mMc[:, c1, :],
                            xqT[:, c1, tt * 512 : (tt + 1) * 512],
                            start=(c1 == 0),
                            stop=(c1 == CC - 1),
                        )
                nc.vector.tensor_copy(out=uT[:, c2, :], in_=ups2[:, :, :])
            # xT loads: sync FIFO right after the mall reads; needed by 5a
            for cc in range(CC):
                nc.sync.dma_start(
                    out=xT[:, cc, :], in_=xT_ext[cc * 128 : (cc + 1) * 128, :]
                )
        xq_p.close()  # xqT freed before 5a opens

        # ======== phase 4 (5a): scores + softmax + rescaled transposed probs =
        with ExitStack() as ph5:
            st5 = ph5.enter_context(tc.tile_pool(name="st5", bufs=2))
            small = ph5.enter_context(tc.tile_pool(name="small", bufs=4))
            ps5 = ph5.enter_context(tc.tile_pool(name="ps5", bufs=1, space="PSUM"))
            ps_t = ph5.enter_context(tc.tile_pool(name="ps_t", bufs=2, space="PSUM"))
            m2 = st5.tile([128, 8, 256], F32, tag="m2", bufs=1)  # 8KB/part
            for k in range(8):
                nc.sync.dma_start(out=m2[:, k, :], in_=m2_ext[k])

            def softmax_stage(k):
                """scores (c2-outer: stationary reused 4x) -> mask -> exp"""
                ek = E[k]
                scols = ek * 128
                sps = ps5.tile([128, 2048], F32, tag="sps", name=f"sps{k}")  # 4 banks
                for c2 in range(CC):
                    for st in range(0, scols, 512):
                        w = min(512, scols - st)
                        nc.tensor.matmul(
                            sps[:, st : st + w],
                            uT[:, c2, k * 128 : (k + 1) * 128],
                            xT[:, c2, st : st + w],
                            start=(c2 == 0),
                            stop=(c2 == CC - 1),
                        )
                s_sb = st5.tile([128, T], F32, tag="s_sb", name=f"s_sb{k}")
                m0 = scols - 256  # mask window start
                if m0 > 0:
                    nc.vector.tensor_copy(out=s_sb[:, :m0], in_=sps[:, :m0])
                nc.vector.tensor_add(s_sb[:, m0:scols], sps[:, m0:scols], m2[:, k, :])
                negmax = small.tile(
                    [128, 1], F32, tag="negmax", name=f"negmax{k}", bufs=6
                )
                nc.vector.tensor_reduce(
                    out=negmax,
                    in_=s_sb[:, :scols],
                    axis=mybir.AxisListType.X,
                    op=mybir.AluOpType.max,
                    negate=True,
                )
                psb = st5.tile([128, T], BF16, tag="psb", name=f"psb{k}", bufs=2)
                rsum = small.tile([128, 1], F32, tag="rsum", name=f"rsum{k}", bufs=6)
                nc.scalar.activation(
                    out=psb[:, :scols],
                    in_=s_sb[:, :scols],
                    func=mybir.ActivationFunctionType.Exp,
                    bias=negmax,
                    scale=1.0,
                    accum_out=rsum,
                )
                rinv = small.tile([128, 1], F32, tag="rinv", name=f"rinv{k}", bufs=6)
                nc.vector.reciprocal(out=rinv, in_=rsum)
                psbS = st5.tile([128, T], BF16, tag="psbS", name=f"psbS{k}", bufs=2)
                nc.vector.tensor_scalar_mul(psbS[:, :scols], psb[:, :scols], rinv)
                return psbS

            def transpose_stage(k, psbS):
                for sc in range(E[k]):
                    pt = ps_t.tile([128, 128], BF16, tag="pt", name=f"pt{k}_{sc}")
                    nc.tensor.transpose(
                        pt[:, :], psbS[:, sc * 128 : (sc + 1) * 128], identbf[:, :]
                    )
                    nc.vector.tensor_copy(out=ptsb[:, EOFF[k] + sc, :], in_=pt[:, :])
                # stream P' tiles to DRAM for the pair exchange (scalar FIFO)
                o = EOFF[k]
                nc.scalar.dma_start(
                    out=pts_d[o : o + E[k]].transpose([1, 0, 2]),
                    in_=ptsb[:, o : o + E[k], :],
                )

            staged = {0: softmax_stage(0)}
            for k in range(8):
                if k + 1 < 8:
                    staged[k + 1] = softmax_stage(k + 1)
                transpose_stage(k, staged.pop(k))
            nc.gpsimd.collective_compute(
                "AllGather",
                mybir.AluOpType.bypass,
                replica_groups=PAIRS,
                ins=[pts_d[:, :, :]],
                outs=[pall_d[:, :, :, :]],
            )

        # ======== phase 5 (vf): vf-half = x[all T] @ N2[:, my g-half] ========
        # runs on PE while the P' exchange is in flight; vf lives in SBUF only
        with ExitStack() as ph4:
            n2p = ph4.enter_context(tc.tile_pool(name="n2p", bufs=1))
            ps4 = ph4.enter_context(tc.tile_pool(name="ps4", bufs=2, space="PSUM"))
            n2h = n2p.tile([128, CC, 1024], BF16, tag="n2h")  # 16KB/part
            # n2h_d rows [pos 4][cc][p][256] -> n2h[p, cc, pos*256:...]
            for gi in range(4):
                nc.sync.dma_start(
                    out=n2h[:, :, gi * 256 : (gi + 1) * 256],
                    in_=n2h_d[gi].transpose([1, 0, 2]),
                )
            xvb = n2p.tile([128, CC, T], BF16, tag="xvb")  # 32KB/part
            for cc in range(CC):
                nc.sync.dma_start(
                    out=xvb[:, cc, :], in_=xvT_ext[cc * 128 : (cc + 1) * 128, :]
                )
            for tb in range(NB):
                vps = ps4.tile([128, 1024], F32, tag="vps", name=f"vps{tb}")
                for cc in range(CC):  # stationary reused across both movings
                    for g2 in range(2):
                        nc.tensor.matmul(
                            vps[:, g2 * 512 : (g2 + 1) * 512],
                            xvb[:, cc, tb * 128 : (tb + 1) * 128],
                            n2h[:, cc, g2 * 512 : (g2 + 1) * 512],
                            start=(cc == 0),
                            stop=(cc == CC - 1),
                        )
                nc.vector.tensor_copy(out=vfo[:, tb, :], in_=vps[:, :])

        # ======== phase 6 (5b): out[:, my g-half] = P'm @ vf, both members ===
        with ExitStack() as ph6:
            prd = ph6.enter_context(tc.tile_pool(name="prd", bufs=2))
            orp = ph6.enter_context(tc.tile_pool(name="orp", bufs=2))
            ps6 = ph6.enter_context(tc.tile_pool(name="ps6", bufs=2, space="PSUM"))
            bfh = orp.tile([128, 1024], F32, tag="bfh", bufs=1)  # 4KB/part
            bfh_ap = bfh_ext[:]
            nc.sync.dma_start(
                out=bfh,
                in_=bass.AP(
                    tensor=bfh_ap.tensor,
                    offset=bfh_ap.offset,
                    ap=[[0, 128]] + list(bfh_ap.ap),
                ),
            )

            def pread(m, k, name):
                e = E[k]
                t = prd.tile([128, 16, 128], BF16, tag="pk", name=name, bufs=4)
                src = pall_d[m, EOFF[k] : EOFF[k] + e]
                nc.sync.dma_start(out=t[:, :e, :], in_=src.transpose([1, 0, 2]))
                return t

            def epilogue(m, k, ops):
                orow = orp.tile([128, 1024], BF16, tag="orow", name=f"or{m}_{k}")
                nc.vector.tensor_add(orow, ops, bfh)
                nc.scalar.dma_start(out=out_ext[m, k], in_=orow)

            for m in range(2):  # pair member whose queries we compute
                for ka, kb in WAVES:
                    ea, eb = E[ka], E[kb]
                    pka = pread(m, ka, f"pka{m}_{ka}")
                    pkb = pread(m, kb, f"pkb{m}_{kb}")
                    opsA = ps6.tile([128, 1024], F32, tag="opsA", name=f"oA{m}{ka}")
                    opsB = ps6.tile([128, 1024], F32, tag="opsB", name=f"oB{m}{kb}")
                    for sc in range(ea):
                        for g2 in range(2):
                            nc.tensor.matmul(
                                opsA[:, g2 * 512 : (g2 + 1) * 512],
                                pka[:, sc, :],
                                vfo[:, sc, g2 * 512 : (g2 + 1) * 512],
                                start=(sc == 0),
                                stop=(sc == ea - 1),
                            )
                        if sc < eb:
                            for g2 in range(2):
                                nc.tensor.matmul(
                                    opsB[:, g2 * 512 : (g2 + 1) * 512],
                                    pkb[:, sc, :],
                                    vfo[:, sc, g2 * 512 : (g2 + 1) * 512],
                                    start=(sc == 0),
                                    stop=(sc == eb - 1),
                                )
                        if sc == eb:
                            epilogue(m, kb, opsB)  # B stopped; drain while A runs
                    epilogue(m, ka, opsA)

    nc.finalize()
    return nc


def _get_program():
    if "nc" not in _CACHE:
        _CACHE["nc"] = _build_program()
    return _CACHE["nc"]


def _make_in_maps(x, Wq, Wk, Wv, Wf, bf):
    x = np.ascontiguousarray(x, dtype=np.float32)
    WqT = np.ascontiguousarray(np.asarray(Wq, dtype=np.float32).T)
    WkT = np.ascontiguousarray(np.asarray(Wk, dtype=np.float32).T)
    WvTb = np.ascontiguousarray(np.asarray(Wv, dtype=np.float32).T).astype(
        ml_dtypes.bfloat16
    )
    Wfb = np.asarray(Wf, dtype=np.float32).astype(ml_dtypes.bfloat16)
    bf = np.ascontiguousarray(bf, dtype=np.float32)
    in_maps = []
    for core in range(NCORES):
        b, h = core // 2, core % 2
        own = OWN_H[h]
        xb = x[b]
        xbT = np.ascontiguousarray(xb.T)
        xq = np.concatenate([xb[blk * 128 : (blk + 1) * 128] for blk in own], axis=0)
        mask2 = np.zeros((8, 128, 256), dtype=np.float32)
        for k, blk in enumerate(own):
            s0 = (E[k] - 2) * 128  # global key index of mask window start
            s = s0 + np.arange(256)[None, :]
            t = blk * 128 + np.arange(128)[:, None]
            mask2[k] = np.where(s <= t, 0.0, NEG).astype(np.float32)
        gidx = h * 4 + b  # my Wf g-slice; group [h::2] holds g-half h in order
        in_maps.append(
            {
                "xTin": xbT,
                "xqTin": np.ascontiguousarray(xq.T),
                "xvTb": xbT.astype(ml_dtypes.bfloat16),
                "mask2": mask2,
                "WkT": WkT,
                "WqTs": np.ascontiguousarray(WqT[:, core * 128 : (core + 1) * 128]),
                "WvTb": WvTb,
                "Wfs": np.ascontiguousarray(Wfb[:, gidx * 256 : (gidx + 1) * 256]),
                "bfh": np.ascontiguousarray(bf[h * 1024 : (h + 1) * 1024]),
            }
        )
    return in_maps


def run_on_hw(inputs, trace=False, trace_cores=None):
    nc = _get_program()
    in_maps = _make_in_maps(**inputs)
    res = run_bass_kernel_spmd(
        nc, in_maps, list(range(NCORES)), trace=trace, trace_cores=trace_cores
    )
    out = np.empty((B, T, F), dtype=np.float32)
    for core in range(NCORES):
        b, h = core // 2, core % 2
        o = res.results[core]["out"]  # [2, 8, 128, 1024] bf16
        for m in range(2):
            for k, blk in enumerate(OWN_H[m]):
                out[b, blk * 128 : (blk + 1) * 128, h * 1024 : (h + 1) * 1024] = o[
                    m, k
                ].astype(np.float32)
    return out, res


def kernel(x, Wq, Wk, Wv, Wf, bf):
    out, _ = run_on_hw(dict(x=x, Wq=Wq, Wk=Wk, Wv=Wv, Wf=Wf, bf=bf))
    return out
